# revision 18
# baseline (speedup 1.0000x reference)
"""Trainium2 Bass kernel for topk_masking (hidden-point-removal style).

Computes, for each of N=16384 points: pairwise scores
  scores[i, j] = <dir_i, tp_j>   (dir = normalized centered points,
                                  tp = ||p||^gamma * dir)
then per-row top-k values (k<=16), and
  w = elu((tpn_i - topk) / (top1 - topk)),  visible = w > 0.99.

Sharding: rows i are split across 8 NeuronCores (2048 rows each); tp is
replicated. Each core computes its 2048 x 16384 score tile on the
TensorEngine in 128x512 PSUM chunks and reduces each 2048-wide PSUM tile
to its top-8 values with the VectorEngine's max8 instruction; the 8*8=64
per-tile candidates per row are then reduced to the row's top-16
(max8 + match_replace + max8). The O(N) prologue (normalize) and
epilogue (elu) run on host.

Precision/speed trick: a plain fp32 matmul runs the PE at 1/4 rate
(LOW_HIGH double pass) and fp32r (full rate) only carries ~11 mantissa
bits, whose error the tiny top1-top10 gaps amplify into w. Instead each
fp32 operand is split into 3 bf16 components (hi/mid/lo residuals) and
the 6 cross-products with magnitude >= 2^-16 are evaluated in ONE bf16
matmul by stacking them along the contraction dim (K=3 -> 18, still one
PE pass at 1 cycle/moving-col). Score error ~3e-7 relative, PE at full
rate.
"""

import numpy as np

import jax
from jax.sharding import Mesh, PartitionSpec
from jax.experimental.shard_map import shard_map

import concourse.mybir as mybir
import concourse.tile as tile
from concourse import bacc
from concourse.bass2jax import _bass_exec_p, install_neuronx_cc_hook

N = 16384
D = 3
NSPLIT = 6               # (hi,hi) (hi,mid) (mid,hi) (hi,lo) (lo,hi) (mid,mid)
DS = D * NSPLIT          # stacked contraction dim = 18
NCORES = 8
R = N // NCORES          # 2048 rows per core
NBLK = R // 128          # 16 row-blocks per core
CHUNK = 512              # moving-operand width per matmul
NCHUNK = N // CHUNK      # 32 column chunks
EPS = 1e-12
GAMMA = -0.5
VIS_THRESH = 0.99
NEG_BIG = -1.0e30

_CACHE = {}


def _build(reps=1, noop=False, wide=2048, psum_bufs=2, mv=CHUNK):
    """Build + compile the SPMD Bass program (same NEFF on all 8 cores).

    reps > 1 unrolls the whole computation `reps` times inside the NEFF so
    (t[reps] - t[1]) / (reps - 1) isolates steady-state per-problem HW time
    from dispatch/transfer overhead. noop=True keeps the same I/O but does
    no compute (measures the dispatch floor).
    """
    nc = bacc.Bacc(
        "TRN2",
        target_bir_lowering=False,
        debug=False,
        enable_asserts=False,
        num_devices=NCORES,
        enable_partition_id=False,
    )
    bf16 = mybir.dt.bfloat16
    NW = N // wide
    dirs_in = nc.dram_tensor("dirs", [DS, R], bf16, kind="ExternalInput").ap()
    tp_in = nc.dram_tensor("tp", [DS, N], bf16, kind="ExternalInput").ap()
    out = nc.dram_tensor(
        "cand", [128, NBLK * NW * 8], mybir.dt.float32, kind="ExternalOutput"
    ).ap()

    with tile.TileContext(nc) as tc:
        with (
            tc.tile_pool(name="const", bufs=1) as const_pool,
            tc.tile_pool(name="psum", bufs=psum_bufs, space="PSUM") as psum_pool,
            tc.tile_pool(name="outp", bufs=1) as out_pool,
        ):
            dirs_sb = const_pool.tile([DS, R], bf16)
            tp_sb = const_pool.tile([DS, N], bf16)
            # dirs on the scalar hwdge queue so it overlaps the first tp slice
            nc.scalar.dma_start(dirs_sb[:], dirs_in)
            # split so the first matmuls only wait on their own column range
            for g in range(NW):
                nc.sync.dma_start(
                    tp_sb[:, g * wide : (g + 1) * wide],
                    tp_in[:, g * wide : (g + 1) * wide],
                )

            out_sb = out_pool.tile([128, NBLK * NW * 8], mybir.dt.float32)

            if noop in (True, "mm", "max"):
                nc.vector.memset(out_sb[:], 0.0)
                nc.sync.dma_start(out, out_sb[:])
            if noop == "mm":
                # PE-only: all matmuls, no DVE consumers.
                for rep in range(reps):
                    for b in range(NBLK):
                        lhsT = dirs_sb[:, b * 128 : (b + 1) * 128]
                        for c in range(N // mv):
                            pt = psum_pool.tile([128, mv], mybir.dt.float32, tag="pt")
                            nc.tensor.matmul(
                                pt[:], lhsT, tp_sb[:, c * mv : (c + 1) * mv],
                                start=True, stop=True,
                            )
            elif noop == "max":
                # DVE-only: max8 over a fixed SBUF chunk, full count.
                fixed = const_pool.tile([128, wide], mybir.dt.float32)
                nc.vector.memset(fixed[:], 1.0)
                for rep in range(reps):
                    for b in range(NBLK):
                        for c in range(N // wide):
                            nc.vector.max(
                                out=out_sb[:, b * NW * 8 + c * 8 : b * NW * 8 + (c + 1) * 8],
                                in_=fixed[:],
                            )
            # Wide scan: PE fills a (128, wide) PSUM tile (wide/512 banks)
            # with bf16 matmuls; DVE reduces it with ONE wide max8 straight
            # from PSUM into the candidate output tile. The top-16-of-64
            # per-row reduction runs on host (off the DVE critical path).
            WIDE = wide
            NWIDE = NW                     # wide chunks per row
            SUB = WIDE // mv               # matmuls per wide chunk
            CB = NWIDE * 8                 # candidate floats per row-block
            for rep in range(0 if noop else reps):
                for b in range(NBLK):
                    lhsT = dirs_sb[:, b * 128 : (b + 1) * 128]
                    for g in range(NWIDE):
                        pt = psum_pool.tile([128, WIDE], mybir.dt.float32, tag="pt")
                        for s in range(SUB):
                            c = g * SUB + s
                            nc.tensor.matmul(
                                pt[:, s * mv : (s + 1) * mv],
                                lhsT,
                                tp_sb[:, c * mv : (c + 1) * mv],
                                start=True,
                                stop=True,
                            )
                        # top-8 of the wide chunk, straight from PSUM
                        nc.vector.max(
                            out=out_sb[:, b * CB + g * 8 : b * CB + (g + 1) * 8],
                            in_=pt[:],
                        )
                    if b % 2 == 1:
                        # stream candidates out as they complete
                        nc.sync.dma_start(
                            out[:, (b - 1) * CB : (b + 1) * CB],
                            out_sb[:, (b - 1) * CB : (b + 1) * CB],
                        )

    nc.compile()
    return nc


def _get_runner(reps=1, noop=False, **cfg):
    """Cached PJRT runner: jitted shard_map over 8 cores, reusable across calls.

    Mimics concourse.bass2jax.run_bass_via_pjrt's multi-core branch, but keeps
    the jitted function so repeated calls don't re-trace. One runner per
    (reps, noop) NEFF variant.
    """
    key = ("runner", reps, noop, tuple(sorted(cfg.items())))
    if key in _CACHE:
        return _CACHE[key]

    nc = _build(reps=reps, noop=noop, **cfg)
    install_neuronx_cc_hook()

    in_names, out_names, out_avals = [], [], []
    for alloc in nc.m.functions[0].allocations:
        if not isinstance(alloc, mybir.MemoryLocationSet):
            continue
        name = alloc.memorylocations[0].name
        if alloc.kind == "ExternalInput":
            in_names.append(name)
        elif alloc.kind == "ExternalOutput":
            out_names.append(name)
            out_avals.append(
                jax.core.ShapedArray(tuple(alloc.tensor_shape), mybir.dt.np(alloc.dtype))
            )
    assert nc.partition_id_tensor is None and nc.dbg_addr is None
    n_params = len(in_names)
    n_outs = len(out_names)
    all_names = in_names + out_names

    def _body(*args):
        outs = _bass_exec_p.bind(
            *args,
            out_avals=tuple(out_avals),
            in_names=tuple(all_names),
            out_names=tuple(out_names),
            lowering_input_output_aliases=(),
            sim_require_finite=True,
            sim_require_nnan=True,
            nc=nc,
        )
        return tuple(outs)

    devices = jax.devices()[:NCORES]
    mesh = Mesh(np.asarray(devices), ("core",))
    donate = tuple(range(n_params, n_params + n_outs))

    jitted = jax.jit(
        shard_map(
            _body,
            mesh=mesh,
            in_specs=(PartitionSpec("core"),) * (n_params + n_outs),
            out_specs=(PartitionSpec("core"),) * n_outs,
            check_rep=False,
        ),
        donate_argnums=donate,
        keep_unused=True,
    )

    def run(per_core_inputs):
        """per_core_inputs: list of NCORES dicts name->array. Returns list of
        NCORES dicts name->np.ndarray."""
        concat_in = [
            np.concatenate([np.asarray(pc[name]) for pc in per_core_inputs], axis=0)
            for name in in_names
        ]
        concat_zero = [
            np.zeros((NCORES * a.shape[0], *a.shape[1:]), a.dtype) for a in out_avals
        ]
        out_arrs = jitted(*concat_in, *concat_zero)
        return [
            {
                name: np.asarray(out_arrs[i]).reshape(
                    NCORES, *out_avals[i].shape
                )[c]
                for i, name in enumerate(out_names)
            }
            for c in range(NCORES)
        ]

    _CACHE[key] = run
    return run


def _host_prep(pts, viewpoint):
    """Mirror of the reference prologue, in fp32 numpy. pts: (3, N)."""
    centered = (pts - viewpoint[:, None]).astype(np.float32)
    norm = np.sqrt(np.sum(centered * centered, axis=0, dtype=np.float32)).astype(
        np.float32
    )
    normc = np.maximum(norm, np.float32(EPS))
    dirs = (centered / normc[None, :]).astype(np.float32)
    tpn = np.power(norm, np.float32(GAMMA)).astype(np.float32)
    tp = (tpn[None, :] * dirs).astype(np.float32)
    return dirs, tp, tpn


def _split3(x):
    """Split fp32 x into 3 bf16 components with x ~ hi + mid + lo."""
    import ml_dtypes

    bf = ml_dtypes.bfloat16
    hi = x.astype(bf)
    r1 = x - hi.astype(np.float32)
    mid = r1.astype(bf)
    lo = (r1 - mid.astype(np.float32)).astype(bf)
    return hi, mid, lo


def _stack_split(a, b):
    """Stacked [18, n_a], [18, n_b] bf16 operands whose K-contraction equals
    the fp32 product a.T @ b up to ~2^-24: the 6 cross-component products
    with magnitude >= 2^-16 laid out along the contraction dim."""
    a1, a2, a3 = _split3(a)
    b1, b2, b3 = _split3(b)
    a_stack = np.concatenate([a1, a1, a2, a1, a3, a2], axis=0)
    b_stack = np.concatenate([b1, b2, b1, b3, b1, b2], axis=0)
    return np.ascontiguousarray(a_stack), np.ascontiguousarray(b_stack)


def _make_in_maps(dirs, tp):
    dirs_s, tp_s = _stack_split(dirs, tp)
    return [
        {
            "dirs": np.ascontiguousarray(dirs_s[:, c * R : (c + 1) * R]),
            "tp": tp_s,
        }
        for c in range(NCORES)
    ]


def _device_topk(in_maps, reps=1, noop=False, **cfg):
    """Returns the (N, 64) per-row candidate values (top-8 of each 2048-wide
    column chunk); the final per-row top-k reduction runs on host."""
    run = _get_runner(reps=reps, noop=noop, **cfg)
    res = run(in_maps)
    if noop:
        return None
    ncand = (N // 2048) * 8
    vals = np.empty((N, ncand), np.float32)
    for c in range(NCORES):
        t = res[c]["cand"]  # (128, NBLK*ncand)
        vals[c * R : (c + 1) * R] = (
            t.reshape(128, NBLK, ncand).transpose(1, 0, 2).reshape(R, ncand)
        )
    return vals


def kernel(pts, viewpoint, k):
    pts = np.asarray(pts, dtype=np.float32)              # (1, 3, N)
    viewpoint = np.asarray(viewpoint, dtype=np.float32)  # (1, 3)
    kk = int(k)
    assert 1 <= kk <= 16, f"k={kk} unsupported (device computes top-16)"
    assert pts.shape == (1, D, N)

    dirs, tp, tpn = _host_prep(pts[0], viewpoint[0])
    vals = _device_topk(_make_in_maps(dirs, tp))  # (N, 64) candidates

    m = vals.shape[1]
    part = np.partition(vals, [m - kk, m - 1], axis=1)
    top1 = part[:, m - 1]
    topk = part[:, m - kk]
    x = ((tpn - topk) / (top1 - topk)).astype(np.float32)
    w = np.where(x > 0, x, np.expm1(x)).astype(np.float32)[None, :]
    visible_mask = w > np.float32(VIS_THRESH)
    return w, visible_mask
